# revision 49
# baseline (speedup 1.0000x reference)
"""Trainium2 Bass kernel for nn_Attention_85005992722686.

Head-sharded tensor-parallel causal attention over 8 NeuronCores.
Core c owns heads {2c, 2c+1}; layernorms are algebraically folded:

  y = softmax(causal((LN(x;g,b) @ Wq) (LN(x;gc,bc) @ Wk)^T / 8)) @ (LN(x) @ Wv) @ Wo

v6 (178.8us TimelineSim, 1.41x over the 252.6us baseline): bf16
datapath + fp8e4 DoubleRow score matmuls, one near-gapless attention
stream; next-chunk projections/v-transposes (and the final Y groups)
are emitted as FILLERS between the jt iterations of each attention
chunk so the in-order PE queue interleaves them into the exp-paced
pipeline instead of serializing at chunk boundaries.

Per core (hd = 128 = 2 heads x 64), per batch b, per 512-token chunk:
  G: bf16 gram matmuls [128,130] per token-tile (128 data + 2 ones cols)
     -> col sums of x, x^2 -> mean; rstd = (var+eps)^-0.5 via DVE-only
     quake-guess + 2 Newton steps (ACT stays exp-only -> single act
     table load; gpsimd pow and ACT Sqrt both fail codegen/tables).
     Stats transposed to rows on PE but EMITTED LATE (emit_G_rows) so
     the in-order PE queue never head-of-line blocks on the DVE chain;
     rstd row partition-broadcast on gpsimd.
  P: qkv projections (bf16 PE, K=1024 over 8 k-tiles) + rank-1 mean
     correction; DVE eviction scales by rstd -> q/k in fp8e4, v bf16;
     q/k folded [128,n]->[64,(2,n)] (dh = 2p+i) by one SBUF->SBUF DMA
     each - cross-partition folds are only expressible as DMAs.
  V: PE-transpose vT -> v natural + ones cols (denominator trick).
  A: S^T = k^T q per 128-j-tile as one fp8 DoubleRow matmul per head
     (K=32 x 2 planes = dh 64; 0.5 cy/col; NOTE K<=64 only - mixing PE
     tile positions inside one PSUM accumulation group is rejected at
     runtime, which kills DR for K=128 grams), merged exp over both
     heads [128,(2,w)] (ACT), tri-mask diag (Pool; gpsimd cannot touch
     PSUM), PV with [v|1] bf16 lhsT accumulating attn^T + denominators
     in PSUM; normalize via DVE recip + Pool partition_broadcast.
  Y: y_partial = attnhat^T^T @ Wo (bf16), DVE-evicted to bf16 staging,
     one DMA per 2 token-tiles (p-major grouped dram layout, host
     unshuffles).

PSUM (8 banks): psS 2x[128,1024] (S pairs / vT) + psPV 2x[66,512]
(attn^T+denom) + psU 2x[128,512] (gram/st/proj/y ring).

DMA notes: transfers serialize on one DMA device (~360B/ns) and SP
dispatch costs ~1.2us/DMA, so everything is batched: 4 chunk DMAs per
x copy, packed statics, single p-major weight load, stat rows on the
ACT queue, b1/wo loads emitted after the first folds so fold DMAs win
the device.
host: y = sum of 8 partial y (f32 accumulation of bf16 partials).
"""
import sys
sys.path.insert(0, '/opt/trn_rl_repo')
import numpy as np
import ml_dtypes
import concourse.bass as bass
import concourse.bacc as bacc
import concourse.tile as tile
from concourse import mybir
from concourse.bass_utils import run_bass_kernel_spmd

F32 = mybir.dt.float32
BF16 = mybir.dt.bfloat16
FP8 = mybir.dt.float8e4
DR = mybir.MatmulPerfMode.DoubleRow
AF = mybir.ActivationFunctionType
ALU = mybir.AluOpType
I32 = mybir.dt.int32

B, N, D = 2, 2048, 1024
H, DH = 16, 64
EPS = 1e-5
NCORES = 8
HD = 128          # head-dim slice per core (2 heads x 64)
KT = D // 128     # 8 k-tiles over model dim
NT = N // 128     # 16 n-tiles
NCH = N // 512    # 4 n-chunks of 512
BLK = 130         # xT block: 128 data cols + 2 ones cols

TRACE = False
TRACE_KWARGS = {}
LAST_RESULTS = None
PHASE_MARKS = []


def _build_program(with_bias):
    nc = bacc.Bacc("TRN2", target_bir_lowering=False, debug=False,
                   num_devices=NCORES)
    # ---------------- dram io ----------------
    xt_d = nc.dram_tensor("xt", [B, 128, KT, NT, BLK], BF16,
                          kind="ExternalInput")
    wqkv_d = nc.dram_tensor("wqkv", [128, KT * 3 * HD], BF16,
                            kind="ExternalInput")
    wo_d = nc.dram_tensor("wo", [HD, D], BF16, kind="ExternalInput")
    # statics packed: [tri | identb | aux(row0) | biasr(row0)]
    SW = 256 + 3 * HD + (3 * HD if with_bias else 0)
    stat_d = nc.dram_tensor("statics", [128, SW], BF16, kind="ExternalInput")
    identf_d = nc.dram_tensor("identf", [128, 128], F32, kind="ExternalInput")
    # y grouped: [b, group of 4 token-tiles, partition, tile-in-group * D]
    y_d = nc.dram_tensor("y", [B, NT // 2, 128, 2 * D], BF16,
                         kind="ExternalOutput")

    with tile.TileContext(nc) as tc:
        with tc.tile_pool(name="wpool", bufs=1) as wpool, \
             tc.tile_pool(name="xpool", bufs=1) as xpool, \
             tc.tile_pool(name="big", bufs=1) as bigp, \
             tc.tile_pool(name="small", bufs=1) as smallp, \
             tc.tile_pool(name="pstrip", bufs=5) as ppool, \
             tc.tile_pool(name="psS", bufs=2, space="PSUM") as psS, \
             tc.tile_pool(name="psPV", bufs=1, space="PSUM") as psPV, \
             tc.tile_pool(name="psU", bufs=2, space="PSUM") as psU:

            # ================= prologue DMAs (all SP queue) =================
            identf_sb = wpool.tile([128, 128], F32, name="identf_sb")
            xt_sb = {}
            for b in range(B):
                xt_sb[b] = xpool.tile([128, KT * NT * BLK], BF16,
                                      name=f"xt{b}")

            def xv(b):
                return xt_sb[b].rearrange("p (k g c) -> p k g c", g=NT, c=BLK)

            stat_sb = wpool.tile([128, SW], BF16, name="stat_sb")
            wq_sb = wpool.tile([128, KT * 3 * HD], BF16, name="wq_sb")
            nc.sync.dma_start(identf_sb[:], identf_d.ap()[:, :])
            nc.sync.dma_start(stat_sb[:], stat_d.ap()[:, :])
            nc.sync.dma_start(wq_sb[:], wqkv_d.ap()[:, :])
            for c4 in range(NCH):        # b0 bf16
                nc.sync.dma_start(xv(0)[:, :, 4 * c4:4 * c4 + 4, :],
                                  xt_d.ap()[0, :, :, 4 * c4:4 * c4 + 4, :])
            tri_sb = stat_sb[:, 0:128]
            identb_sb = stat_sb[:, 128:256]
            aux_sb = stat_sb[0:1, 256:256 + 3 * HD]
            if with_bias:
                bias_sb = stat_sb[0:1, 256 + 3 * HD:SW]
            w_sb = {}
            for kt in range(KT):
                for ti in range(3):
                    w_sb[ti, kt] = wq_sb[:, kt * 3 * HD + ti * HD:
                                         kt * 3 * HD + (ti + 1) * HD]
            wo_sb = wpool.tile([HD, D], BF16, name="wo_sb")

            def emit_late_loads():
                # b1 loads + wo: issued after the first folds so those win
                # the (serialized) DMA engine
                nc.sync.dma_start(wo_sb[:], wo_d.ap()[:, :])
                for c4 in range(NCH):
                    nc.sync.dma_start(xv(1)[:, :, 4 * c4:4 * c4 + 4, :],
                                      xt_d.ap()[1, :, :, 4 * c4:4 * c4 + 4, :])

            # ================= per-b tiles =================
            mean_st = {}; mean_row = {}; s_row = {}; std_row = {}
            qk8u = {}; qk8f = {}; vT = {}; v_sb = {}; attnhat = {}
            for b in range(B):
                # per chunk c: cols [12c:12c+4]=mean, [+4:+8]=rstd,
                # [+8:+12]=E[x^2] -> var -> (std if bias)
                mean_st[b] = smallp.tile([128, 48], F32, name=f"mst{b}",
                                         tag="mst", bufs=2)
                mean_row[b] = smallp.tile([1, N], BF16, name=f"mrow{b}",
                                          tag="mrow", bufs=2)
                s_row[b] = smallp.tile([1, N], BF16, name=f"srow{b}",
                                       tag="srow", bufs=2)
                if with_bias:
                    std_row[b] = smallp.tile([1, N], BF16, name=f"drow{b}",
                                             tag="drow", bufs=2)
                for nm in ("q", "k"):
                    qk8u[b, nm] = bigp.tile([HD, N], FP8, name=f"{nm}8u{b}",
                                            tag=f"{nm}8u", bufs=1)
                    qk8f[b, nm] = bigp.tile([64, 2 * N], FP8,
                                            name=f"{nm}8f{b}",
                                            tag=f"{nm}8f", bufs=2)
                vT[b] = bigp.tile([HD, N], BF16, name=f"vT{b}",
                                  tag="vT", bufs=2)
                v_sb[b] = bigp.tile([128, NT * 132], BF16, name=f"vnat{b}",
                                    tag="vnat", bufs=2)
                attnhat[b] = bigp.tile([HD, N], BF16, name=f"ah{b}",
                                       tag="ah", bufs=2)
            sbc = {}   # rstd broadcast per (b, c4)

            # ================= phase emitters =================
            def emit_G(b, c4):
                """fp8 DoubleRow gram + stats (rstd via gpsimd pow), row
                transpose, row DMAs, rstd broadcast; per-chunk pipelined."""
                cm = mean_st[b][:, 12 * c4:12 * c4 + 4]
                cr = mean_st[b][:, 12 * c4 + 4:12 * c4 + 8]
                cd = mean_st[b][:, 12 * c4 + 8:12 * c4 + 12]
                g_tiles = []
                for i4 in range(4):
                    blk = 4 * c4 + i4
                    g_ps = psU.tile([128, 512], F32, name=f"g{b}_{c4}_{i4}",
                                    tag="u", bufs=2)
                    for kt in range(KT):
                        nc.tensor.matmul(
                            g_ps[:, 0:BLK],
                            xv(b)[:, kt, blk, 0:128],
                            xv(b)[:, kt, blk, 0:BLK],
                            start=(kt == 0), stop=(kt == KT - 1))
                    g_tiles.append((g_ps, i4))
                scratch = smallp.tile([128, 128], F32, name=f"scr{b}_{c4}",
                                      tag="scr", bufs=2)
                for g_ps, i4 in g_tiles:
                    nc.vector.scalar_tensor_tensor(
                        out=scratch[:],
                        in0=g_ps[:, 0:128], scalar=1.0 / D,
                        in1=identf_sb[:],
                        op0=ALU.mult, op1=ALU.mult,
                        accum_out=cd[:, i4:i4 + 1])
                    nc.vector.tensor_scalar(
                        out=cm[:, i4:i4 + 1],
                        in0=g_ps[:, 128:129], scalar1=1.0 / D, scalar2=None,
                        op0=ALU.mult)
                sq = smallp.tile([128, 4], F32, name=f"sq{b}_{c4}", tag="sq",
                                 bufs=2)
                nc.vector.tensor_mul(sq[:], cm, cm)
                nc.vector.scalar_tensor_tensor(
                    out=cd, in0=cd, scalar=EPS, in1=sq[:],
                    op0=ALU.add, op1=ALU.subtract)
                # rstd = (var+eps)^-0.5 on DVE: quake initial guess + 2
                # Newton steps (rel err ~5e-6); keeps ACT exp-only.
                yi = smallp.tile([128, 4], I32, name=f"yi{b}_{c4}", tag="yi",
                                 bufs=2)
                nc.vector.tensor_scalar(out=yi[:], in0=cd.bitcast(I32),
                                        scalar1=1, scalar2=None,
                                        op0=ALU.arith_shift_right)
                nc.vector.tensor_scalar(out=yi[:], in0=yi[:], scalar1=-1.0,
                                        scalar2=float(0x5f3759df),
                                        op0=ALU.mult, op1=ALU.add)
                y0 = yi[:].bitcast(F32)
                for it_ in range(2):
                    ysrc = y0 if it_ == 0 else cr
                    nc.vector.tensor_mul(sq[:], ysrc, ysrc)
                    nc.vector.scalar_tensor_tensor(
                        out=sq[:], in0=sq[:], scalar=-0.5, in1=cd,
                        op0=ALU.mult, op1=ALU.mult)
                    nc.vector.tensor_scalar(out=sq[:], in0=sq[:],
                                            scalar1=1.5, scalar2=None,
                                            op0=ALU.add)
                    nc.vector.tensor_mul(cr, ysrc, sq[:])
                if with_bias:
                    nc.vector.tensor_mul(cd, cd, cr)   # std = var * rstd
            def emit_G_rows(b, c4):
                """stats cols -> rows (PE transpose, emitted late so the
                DVE newton chain never head-of-line blocks PE), row DMAs,
                rstd broadcast."""
                cr = mean_st[b][:, 12 * c4 + 4:12 * c4 + 8]
                st_ps = psU.tile([128, 512], F32, name=f"stp{b}_{c4}",
                                 tag="u", bufs=2)
                nc.tensor.transpose(st_ps[0:12, 0:128],
                                    mean_st[b][:, 12 * c4:12 * c4 + 12],
                                    identf_sb[:])
                st_bf = smallp.tile([12, 128], BF16, name=f"stb{b}_{c4}",
                                    tag="stb", bufs=2)
                nc.vector.tensor_copy(st_bf[:], st_ps[0:12, 0:128])
                sl = slice(c4 * 512, (c4 + 1) * 512)
                nc.scalar.dma_start(mean_row[b][0:1, sl], st_bf[0:4, :])
                nc.scalar.dma_start(s_row[b][0:1, sl], st_bf[4:8, :])
                if with_bias:
                    nc.scalar.dma_start(std_row[b][0:1, sl], st_bf[8:12, :])
                t = smallp.tile([128, 512], BF16, name=f"sbc{b}_{c4}",
                                tag="sbc", bufs=4)
                nc.gpsimd.partition_broadcast(t[:], s_row[b][0:1, sl])
                sbc[b, c4] = t

            def emit_P_one(b, c4, ti):
                """one qkv projection tensor for chunk c4 + rank-1 mean
                correction; rstd-scaled eviction: q/k -> fp8 (then
                DMA-folded to the DoubleRow layout), v -> bf16."""
                sl = slice(c4 * 512, (c4 + 1) * 512)
                nm = ("q", "k", "v")[ti]
                pr = psU.tile([128, 512], F32, name=f"pr{b}{nm}{c4}",
                              tag="u", bufs=2)
                for kt in range(KT):
                    nc.tensor.matmul(
                        pr[:], w_sb[ti, kt],
                        xv(b)[:, kt, 4 * c4:4 * c4 + 4, 0:128],
                        start=(kt == 0), stop=False)
                nc.tensor.matmul(
                    pr[:], aux_sb[0:1, ti * HD:(ti + 1) * HD],
                    mean_row[b][0:1, sl],
                    start=False, stop=not with_bias)
                if with_bias:
                    nc.tensor.matmul(
                        pr[:], bias_sb[0:1, ti * HD:(ti + 1) * HD],
                        std_row[b][0:1, sl],
                        start=False, stop=True)
                dst = vT[b] if nm == "v" else qk8u[b, nm]
                with nc.allow_low_precision(reason="fp8 scores"):
                    nc.vector.tensor_mul(dst[:, sl], pr[:], sbc[b, c4][:])
                if nm != "v":
                    # fold [128, n] -> [64, (2, n)] with dh_local = 2p + i:
                    # src partition order == dst (p, i) order -> one DMA
                    f = qk8f[b, nm].rearrange("p (i n) -> p i n", i=2)
                    nc.sync.dma_start(f[0:64, :, sl], qk8u[b, nm][:, sl])

            def emit_P(b, c4):
                for ti in range(3):
                    emit_P_one(b, c4, ti)

            def emit_Vones(b):
                vv = v_sb[b].rearrange("p (n u c) -> p n u c", u=2, c=66)
                nc.gpsimd.memset(vv[:, :, :, 64:66], 1.0)

            def emit_Vg(b, g):
                """vT -> v natural layout (chunk g) via psS ring, bitcast
                to bf16."""
                vv = v_sb[b].rearrange("p (n u c) -> p n u c", u=2, c=66)
                vt_ps = psS.tile([128, 1024], F32, name=f"vt{b}_{g}",
                                 tag="s", bufs=2)
                vb = vt_ps.bitcast(BF16)
                for j in range(4):
                    nt = 4 * g + j
                    nc.tensor.transpose(
                        vb[:, j * 128:(j + 1) * 128],
                        vT[b][:, nt * 128:(nt + 1) * 128],
                        identb_sb)
                vsrc = vb[:, 0:512].rearrange("p (n u c) -> p n u c",
                                              u=2, c=64)
                dst = vv[:, 4 * g:4 * g + 4, :, 0:64]
                nc.vector.tensor_copy(dst, vsrc)

            def v_aug(b, jt, h):
                return v_sb[b][:, jt * 132 + h * 66: jt * 132 + (h + 1) * 66]

            def emit_A(b, c4, fillers=()):
                """attention rows chunk c4: fp8 DoubleRow S^T blocks,
                merged exp, diag mask, PV accumulate, normalize. Filler
                closures are emitted between jts so the in-order PE queue
                interleaves other phases into the exp-paced pipeline."""
                fillers = list(fillers)
                sl = slice(c4 * 512, (c4 + 1) * 512)
                pv_ps = [psPV.tile([66, 512], F32, name=f"pv{b}{c4}_{h}",
                                   tag=f"pv{h}", bufs=1) for h in range(2)]
                njt = 4 * c4 + 4
                for jt in range(njt):
                    off = 0 if jt < 4 * c4 else (jt - 4 * c4) * 128
                    w = 512 - off
                    s_ps = psS.tile([128, 1024], F32, name=f"s{b}{c4}_{jt}",
                                    tag="s", bufs=2)
                    kf = qk8f[b, "k"].rearrange("p (i n) -> p i n", i=2)
                    qf = qk8f[b, "q"].rearrange("p (i n) -> p i n", i=2)
                    for h in range(2):
                        hp = slice(32 * h, 32 * h + 32)
                        nc.tensor.matmul(
                            s_ps[:, h * 512 + off:(h + 1) * 512],
                            kf[hp, :, jt * 128:(jt + 1) * 128],
                            qf[hp, :, c4 * 512 + off:(c4 + 1) * 512],
                            start=True, stop=True, skip_group_check=True,
                            perf_mode=DR)
                    p_sb = ppool.tile([128, 1024], BF16,
                                      name=f"p{b}{c4}_{jt}", tag="p", bufs=5)
                    sv = s_ps.rearrange("p (h c) -> p h c", c=512)
                    pv = p_sb.rearrange("p (h c) -> p h c", c=512)
                    nc.scalar.activation(pv[:, :, off:512], sv[:, :, off:512],
                                         AF.Exp)
                    if off > 0 or jt == 4 * c4:
                        # diagonal block: mask j>i within [off:off+128)
                        for h in range(2):
                            nc.gpsimd.tensor_mul(pv[:, h, off:off + 128],
                                                 pv[:, h, off:off + 128],
                                                 tri_sb)
                    for h in range(2):
                        nc.tensor.matmul(pv_ps[h][:, off:512], v_aug(b, jt, h),
                                         pv[:, h, off:512],
                                         start=(jt == 0), stop=(jt == njt - 1))
                    if fillers and jt % 2 == 1:
                        fillers.pop(0)()
                for f_ in fillers:
                    f_()
                # normalize: attnhat[64h:64h+64, sl] = attn / denom
                for h in range(2):
                    rd = smallp.tile([1, 512], BF16, name=f"rd{b}{c4}{h}",
                                     tag="rd", bufs=2)
                    with nc.allow_low_precision(reason="softmax denominators"):
                        nc.vector.reciprocal(rd[:], pv_ps[h][64:65, :])
                    rb = smallp.tile([64, 512], BF16, name=f"rb{b}{c4}{h}",
                                     tag="rb", bufs=4)
                    nc.gpsimd.partition_broadcast(rb[:], rd[:])
                    nc.vector.tensor_mul(
                        attnhat[b][h * 64:(h + 1) * 64, sl],
                        pv_ps[h][0:64, :], rb[:])

            def emit_Y_half(b, g, half):
                """out projection for 2 token-tiles, one grouped DMA."""
                if True:
                    y_sb = smallp.tile([128, 2 * D], BF16,
                                       name=f"y{b}_{g}_{half}",
                                       tag="ysb", bufs=2)
                    for j in range(2):
                        it = 4 * g + 2 * half + j
                        for e in range(2):
                            y_ps = psU.tile([128, 512], F32,
                                            name=f"yp{b}{it}{e}",
                                            tag="u", bufs=2)
                            nc.tensor.matmul(
                                y_ps[:],
                                attnhat[b][:, it * 128:(it + 1) * 128],
                                wo_sb[:, e * 512:(e + 1) * 512],
                                start=True, stop=True)
                            dst = y_sb[:, j * D + e * 512:
                                       j * D + (e + 1) * 512]
                            nc.vector.tensor_copy(dst, y_ps[:])
                    nc.sync.dma_start(y_d.ap()[b, 2 * g + half, :, :],
                                      y_sb[:])

            def emit_Y(b, g):
                emit_Y_half(b, g, 0)
                emit_Y_half(b, g, 1)

            def mark(label):
                PHASE_MARKS.append((label, int(nc.get_next_instruction_name()[2:])))

            # ================= emission schedule =================
            mark("G0c0"); emit_G(0, 0)
            mark("G0c1"); emit_G(0, 1)
            emit_Vones(0); emit_Vones(1)
            mark("G0r0"); emit_G_rows(0, 0)
            mark("G0c2"); emit_G(0, 2)
            mark("G0r1"); emit_G_rows(0, 1)
            mark("G0c3"); emit_G(0, 3)
            mark("G0r2"); emit_G_rows(0, 2)
            mark("P0c0"); emit_P(0, 0); mark("V0g0"); emit_Vg(0, 0)
            mark("G0r3"); emit_G_rows(0, 3)
            mark("P0c1"); emit_P(0, 1); mark("V0g1"); emit_Vg(0, 1)
            mark("lateDMA"); emit_late_loads()
            F = lambda fn, *a: (lambda: fn(*a))
            mark("A0c0"); emit_A(0, 0, [
                F(emit_P_one, 0, 2, 0), F(emit_P_one, 0, 2, 1)])
            mark("P0c2v"); emit_P_one(0, 2, 2); mark("V0g2"); emit_Vg(0, 2)
            mark("A0c1"); emit_A(0, 1, [
                F(emit_P_one, 0, 3, 0), F(emit_P_one, 0, 3, 1),
                F(emit_P_one, 0, 3, 2), F(emit_Vg, 0, 3)])
            mark("Y0g0"); emit_Y(0, 0)
            mark("A0c2"); emit_A(0, 2)
            mark("G1c0"); emit_G(1, 0)
            mark("G1c1"); emit_G(1, 1)
            mark("G1r0"); emit_G_rows(1, 0)
            mark("Y0g1"); emit_Y(0, 1)
            mark("G1r1"); emit_G_rows(1, 1)
            mark("A0c3"); emit_A(0, 3, [
                F(emit_P_one, 1, 0, 0), F(emit_P_one, 1, 0, 1),
                F(emit_P_one, 1, 0, 2), F(emit_Vg, 1, 0)])
            mark("G1c2"); emit_G(1, 2)
            mark("G1c3"); emit_G(1, 3)
            mark("G1r2"); emit_G_rows(1, 2)
            mark("G1r3"); emit_G_rows(1, 3)
            mark("A1c0"); emit_A(1, 0, [
                F(emit_P_one, 1, 1, 0), F(emit_P_one, 1, 1, 1)])
            mark("P1c1v"); emit_P_one(1, 1, 2); mark("V1g1"); emit_Vg(1, 1)
            mark("Y0g2"); emit_Y(0, 2)
            mark("A1c1"); emit_A(1, 1, [
                F(emit_P_one, 1, 2, 0), F(emit_P_one, 1, 2, 1),
                F(emit_P_one, 1, 2, 2), F(emit_Vg, 1, 2)])
            mark("Y0g3"); emit_Y(0, 3)
            mark("A1c2"); emit_A(1, 2, [
                F(emit_P_one, 1, 3, 0), F(emit_P_one, 1, 3, 1),
                F(emit_P_one, 1, 3, 2), F(emit_Vg, 1, 3)])
            mark("A1c3"); emit_A(1, 3, [
                F(emit_Y_half, 1, 0, 0), F(emit_Y_half, 1, 0, 1),
                F(emit_Y_half, 1, 1, 0), F(emit_Y_half, 1, 1, 1)])
            mark("Y1g2"); emit_Y(1, 2)
            mark("Y1g3"); emit_Y(1, 3)
            mark("END")

    nc.compile()
    return nc


_PROG_CACHE = {}


def _get_program(with_bias):
    key = with_bias
    if key not in _PROG_CACHE:
        _PROG_CACHE[key] = _build_program(with_bias)
    return _PROG_CACHE[key]


def kernel(x, ln_g, ln_b, lnc_g, lnc_b, Wq, Wkv, Wo):
    global LAST_RESULTS
    x = np.ascontiguousarray(np.asarray(x, dtype=np.float32))
    ln_g = np.asarray(ln_g, np.float32); ln_b = np.asarray(ln_b, np.float32)
    lnc_g = np.asarray(lnc_g, np.float32); lnc_b = np.asarray(lnc_b, np.float32)
    Wq = np.asarray(Wq, np.float32); Wkv = np.asarray(Wkv, np.float32)
    Wo = np.asarray(Wo, np.float32)
    scale = DH ** -0.5

    with_bias = bool(np.any(ln_b) or np.any(lnc_b))
    nc = _get_program(with_bias)

    # xT p-major: bf16 [B, 128, KT, NT, 130] and fp8 kt-pair-interleaved
    # [B, 128, KT//2, NT, 2, 130], both with ones cols
    xTt = np.transpose(x, (0, 2, 1)).reshape(B, KT, 128, NT, 128)
    xTt = np.transpose(xTt, (0, 2, 1, 3, 4))        # [B, 128, KT, NT, 128]
    xt = np.empty((B, 128, KT, NT, BLK), ml_dtypes.bfloat16)
    xt[..., 0:128] = xTt.astype(ml_dtypes.bfloat16)
    xt[..., 128:130] = 1.0

    tri = np.triu(np.ones((128, 128), np.float32))       # keep col >= row
    ident = np.eye(128, dtype=np.float32)

    in_maps = []
    for c in range(NCORES):
        cs = slice(c * HD, (c + 1) * HD)
        Wq_eff = ln_g[:, None] * Wq[:, cs] * scale
        Wk_eff = lnc_g[:, None] * Wkv[:, :H * DH][:, cs]
        Wv_eff = lnc_g[:, None] * Wkv[:, H * DH:][:, cs]
        wqkv = np.concatenate([Wq_eff, Wk_eff, Wv_eff], axis=1)
        aux = np.concatenate([-Wq_eff.sum(0), -Wk_eff.sum(0),
                              -Wv_eff.sum(0)])
        SW = 256 + 3 * HD + (3 * HD if with_bias else 0)
        statics = np.zeros((128, SW), np.float32)
        statics[:, 0:128] = tri
        statics[:, 128:256] = ident
        statics[0, 256:256 + 3 * HD] = aux
        if with_bias:
            br = np.concatenate([ln_b @ Wq[:, cs] * scale,
                                 lnc_b @ Wkv[:, :H * DH][:, cs],
                                 lnc_b @ Wkv[:, H * DH:][:, cs]])
            statics[0, 256 + 3 * HD:SW] = br
        # w p-major: [128, KT*3*HD], row p col (kt, ti*HD+j) = wqkv[kt*128+p, ...]
        wpm = np.ascontiguousarray(
            wqkv.reshape(KT, 128, 3 * HD).transpose(1, 0, 2).reshape(
                128, KT * 3 * HD))
        m = {
            "xt": xt,
            "wqkv": wpm.astype(ml_dtypes.bfloat16),
            "wo": np.ascontiguousarray(Wo[cs, :]).astype(ml_dtypes.bfloat16),
            "statics": statics.astype(ml_dtypes.bfloat16),
            "identf": ident,
        }
        in_maps.append(m)

    res = run_bass_kernel_spmd(nc, in_maps, core_ids=list(range(NCORES)),
                               trace=TRACE, **TRACE_KWARGS)
    LAST_RESULTS = res
    y = np.zeros((B, NT // 2, 128, 2, D), np.float32)
    for c in range(NCORES):
        y += res.results[c]["y"].reshape(B, NT // 2, 128, 2, D)
    # [b, g, p, j, d] -> token (2g+j)*128+p
    y = np.transpose(y, (0, 1, 3, 2, 4)).reshape(B, N, D)
    return y
